# revision 1
# baseline (speedup 1.0000x reference)
"""ConvAttention Trainium2 kernel (self-contained), v5.

Math: scores[b,h,w,t,s] = convQ(Q)[...,s] + convK(K)[...,t] + b2, softmax over t.
All t-independent terms (A_q, b2, conv bias, boundary terms) cancel in the
softmax, so
  attn[b,h,w,t] = softmax_t(conv5x5(w1k x)[b,h,w,t])
  out[b,c,h,w,s] = sum_t attn[t] (wv x_t + bv)   (indep of s; sum_t attn = 1)

Device pipeline per core (1 batch, bf16 data path):
  stage A (c-major x):  m = u2 @ x (48 mm) -> dram_m -> 50 fanout reads
    -> d2 [(dh%2,h) part, (dh//2,dw,w,t) free] -> conv (30 mm, K=128)
    -> softmax over t -> p [64=(hb,h'), (w,t)] -> dram_p
  stage B (hw-major x2 [128=(h,wb), (w',c,t)]):  p_hw [128,(w',t)] aligns
    with partitions - NO broadcast. pw = x2 * p_hw (c-axis stride-0 bcast),
    q = sum_t pw (dense DVE reduce), 16 PE transposes of q (w'-pair, c)
    tiles, y = wvp @ qT (16 mm N=128), +bv drain.
Output y [128=(w'a,c), 2048=(jj,h,wb)] fp32; host broadcasts s + descrambles.
"""
import numpy as np
from contextlib import ExitStack

B, C, H, W, S = 8, 64, 64, 64, 12
K5 = 5
HB, HH = 2, 32
FREE = HH * W * S          # 24576 free cols per partition
NT = 25                    # taps
WT = W * S                 # 768
NM = 48                    # m matmuls (N=512)
NMC = FREE // NM           # 512
KD = 15                    # d2 free blocks
WP = 32                    # w' per wb block
NCH = 4                    # stage-B chunks (8 w' each)
CHX = FREE // NCH          # 6144 cols (8 w' x 64 c x 12 t)
NTL = 16                   # transpose tiles (w'-pairs)

_cache = {}


def _build_program(debug=False):
    import concourse.bass as bass
    import concourse.tile as tile
    from concourse import bacc, mybir
    f32 = mybir.dt.float32
    bf16 = mybir.dt.bfloat16

    nc = bacc.Bacc("TRN2", target_bir_lowering=False, debug=False, num_devices=8)
    x_in = nc.dram_tensor("x", [128, FREE], bf16, kind="ExternalInput")
    x2_in = nc.dram_tensor("x2", [128, FREE], bf16, kind="ExternalInput")
    u2_in = nc.dram_tensor("u2", [128, 50], bf16, kind="ExternalInput")
    s5_in = nc.dram_tensor("s5", [128, 3 * 64], bf16, kind="ExternalInput")
    id_in = nc.dram_tensor("ident", [128, 128], f32, kind="ExternalInput")
    wvp_in = nc.dram_tensor("wvp", [128, 128], bf16, kind="ExternalInput")
    bvp_in = nc.dram_tensor("bvp", [128, 1], f32, kind="ExternalInput")
    y_out = nc.dram_tensor("y", [128, NTL * 128], f32, kind="ExternalOutput")
    if debug:
        dbg_p = nc.dram_tensor("dbg_p", [64, WT], bf16, kind="ExternalOutput")
        dbg_ph = nc.dram_tensor("dbg_ph", [128, WP * S], bf16,
                                kind="ExternalOutput")
        dbg_q = nc.dram_tensor("dbg_q", [128, NTL * 128], f32,
                               kind="ExternalOutput")

    def ring(i):
        return nc.sync if i % 2 == 0 else nc.scalar

    with tile.TileContext(nc) as tc:
        with ExitStack() as ctx:
            cpool = ctx.enter_context(tc.tile_pool(name="consts", bufs=1))
            u2 = cpool.tile([128, 50], bf16)
            nc.sync.dma_start(u2[:], u2_in[:])
            s5 = cpool.tile([128, 3 * 64], bf16)
            nc.scalar.dma_start(s5[:], s5_in[:])
            ident = cpool.tile([128, 128], f32)
            nc.sync.dma_start(ident[:], id_in[:])
            wvp = cpool.tile([128, 128], bf16)
            nc.scalar.dma_start(wvp[:], wvp_in[:])
            bvp = cpool.tile([128, 1], f32)
            nc.sync.dma_start(bvp[:], bvp_in[:])

            xpool = ctx.enter_context(tc.tile_pool(name="x", bufs=1))
            x_sb = xpool.tile([128, FREE], bf16)
            x2_sb = xpool.tile([128, FREE], bf16)
            for j in range(12):
                sl = bass.ts(j, FREE // 12)
                ring(j).dma_start(x_sb[:, sl], x_in[:, sl])
            for j in range(12):
                sl = bass.ts(j, FREE // 12)
                ring(j).dma_start(x2_sb[:, sl], x2_in[:, sl])

            ppool = ctx.enter_context(tc.tile_pool(name="p", bufs=1))
            p_sb = ppool.tile([64, WT], bf16)
            p_hw = ppool.tile([128, WP * S], bf16)
            ypool = ctx.enter_context(tc.tile_pool(name="y", bufs=1))
            y_sb = ypool.tile([128, NTL * 128], f32)
            q_sb = ypool.tile([128, NTL * 128], f32)
            drpool = ctx.enter_context(
                tc.tile_pool(name="dram", bufs=1, space="DRAM"))
            dram_m = drpool.tile([50, FREE], bf16)
            dram_p = drpool.tile([64, WT], bf16)

            # ---- stage A: tap maps -> fanout transpose -> conv -> softmax ----
            with ExitStack() as actx:
                mpool = actx.enter_context(tc.tile_pool(name="m", bufs=1))
                m_sb = mpool.tile([50, FREE], bf16)
                psM = actx.enter_context(
                    tc.tile_pool(name="psM", bufs=4, space="PSUM"))
                for j in range(NM):
                    ps = psM.tile([50, NMC], f32)
                    nc.tensor.matmul(ps[:], u2[:], x_sb[:, bass.ts(j, NMC)],
                                     start=True, stop=True)
                    dst = m_sb[:, bass.ts(j, NMC)]
                    if j % 2 == 0:
                        nc.scalar.activation(
                            dst, ps[:], mybir.ActivationFunctionType.Identity)
                    else:
                        nc.vector.tensor_copy(dst, ps[:])
                    if j % 6 == 5:
                        sl = bass.ts(j // 6, 6 * NMC)
                        ring(j // 6).dma_start(dram_m[:, sl], m_sb[:, sl])

                dpool = actx.enter_context(tc.tile_pool(name="d2", bufs=1))
                d2 = dpool.tile([128, KD * WT], bf16)
                nc.vector.memset(d2[64:128, 10 * WT:15 * WT], 0.0)
                # dram_m row (hb, dh, dw) [(h', w, t)]
                #  -> d2 partitions (dh%2)*64 + hb*32 + h', free (dh//2,dw,w,t)
                for row in range(50):
                    hb, r = row // NT, row % NT
                    dh, dw = r // 5, r % 5
                    dhp, k = dh % 2, dh // 2
                    src = dram_m[row:row + 1, :].rearrange(
                        "p (hp f) -> p hp f", hp=HH)
                    dst = d2[dhp * 64 + hb * HH: dhp * 64 + (hb + 1) * HH,
                             (k * 5 + dw) * WT:(k * 5 + dw + 1) * WT]
                    ring(row).dma_start(dst, src)

                psA = actx.enter_context(
                    tc.tile_pool(name="psA", bufs=2, space="PSUM"))
                smp = actx.enter_context(tc.tile_pool(name="smax", bufs=2))
                for wh in range(2):
                    a_ps = psA.tile([64, 384], f32)
                    # (k=0, dw=2) first: full w-range, resets the PSUM bank
                    order = [(k, dw) for k in range(3)
                             for dw in [2, 0, 1, 3, 4]]
                    for i, (k, dw) in enumerate(order):
                        lo = max(wh * 32, 2 - dw)
                        hi = min(wh * 32 + 32, 66 - dw)
                        base = (k * 5 + dw) * WT
                        rhs = d2[:, base + (lo + dw - 2) * S:
                                 base + (hi + dw - 2) * S]
                        out = a_ps[:, (lo - wh * 32) * S:(hi - wh * 32) * S]
                        nc.tensor.matmul(out, s5[:, bass.ts(k, 64)], rhs,
                                         start=(i == 0), stop=(i == 14))
                    e_sb = smp.tile([64, 384], bf16)
                    nc.scalar.activation(e_sb[:], a_ps[:],
                                         mybir.ActivationFunctionType.Exp)
                    e3 = e_sb[:].rearrange("p (w t) -> p w t", t=S)
                    z = smp.tile([64, 32], f32)
                    nc.vector.tensor_reduce(z[:], e3, axis=mybir.AxisListType.X,
                                            op=mybir.AluOpType.add)
                    rcp = smp.tile([64, 32], f32)
                    nc.vector.reciprocal(rcp[:], z[:])
                    nc.vector.tensor_mul(
                        p_sb[:, bass.ts(wh, 384)].rearrange(
                            "p (w t) -> p w t", t=S),
                        e3, rcp[:].broadcast_to([64, 32, S]))
            if debug:
                nc.sync.dma_start(dbg_p[:], p_sb[:])

            nc.sync.dma_start(dram_p[:], p_sb[:])
            # p_hw[(wb, h), (w', t)] <- dram_p[h, (w, t)], w = wb*32 + w'
            for wb in range(2):
                ring(wb).dma_start(
                    p_hw[wb * 64:(wb + 1) * 64, :],
                    dram_p[:, wb * WP * S:(wb + 1) * WP * S])
            if debug:
                nc.sync.dma_start(dbg_ph[:], p_hw[:])

            # ---- stage B: aligned multiply, t-reduce, transpose, channel mix
            with ExitStack() as bctx:
                pwp = bctx.enter_context(tc.tile_pool(name="pw", bufs=2))
                psT = bctx.enter_context(
                    tc.tile_pool(name="psT", bufs=4, space="PSUM"))
                psY = bctx.enter_context(
                    tc.tile_pool(name="psY", bufs=4, space="PSUM"))
                qtp = bctx.enter_context(tc.tile_pool(name="qt", bufs=4))
                for ch in range(NCH):
                    pw = pwp.tile([128, CHX], bf16)   # (w' 8, c 64, t 12)
                    pw4 = pw[:].rearrange("p (w c t) -> p w c t", w=8, t=S)
                    x4 = x2_sb[:, bass.ts(ch, CHX)].rearrange(
                        "p (w c t) -> p w c t", w=8, t=S)
                    p4 = p_hw[:, ch * 8 * S:(ch + 1) * 8 * S].rearrange(
                        "p (w o t) -> p w o t", o=1, t=S).broadcast_to(
                        [128, 8, 64, S])
                    nc.vector.tensor_mul(pw4, x4, p4)
                    # q[:, ch block] = sum_t pw  (dense innermost-t reduce)
                    qv = q_sb[:, bass.ts(ch, 512)]
                    nc.vector.tensor_reduce(
                        qv.rearrange("p (w c) -> p w c", w=8),
                        pw4, axis=mybir.AxisListType.X, op=mybir.AluOpType.add)
                    for i in range(4):
                        jj = ch * 4 + i
                        t_ps = psT.tile([128, 128], f32)
                        nc.tensor.transpose(
                            t_ps[:], q_sb[:, jj * 128:(jj + 1) * 128],
                            ident[:])
                        qt = qtp.tile([128, 128], bf16)
                        nc.scalar.activation(
                            qt[:], t_ps[:],
                            mybir.ActivationFunctionType.Identity)
                        y_ps = psY.tile([128, 128], f32)
                        nc.tensor.matmul(y_ps[:], wvp[:], qt[:],
                                         start=True, stop=True)
                        nc.scalar.activation(
                            y_sb[:, jj * 128:(jj + 1) * 128], y_ps[:],
                            mybir.ActivationFunctionType.Identity,
                            bias=bvp[:, 0:1])
                if debug:
                    nc.sync.dma_start(dbg_q[:], q_sb[:])
            nc.sync.dma_start(y_out[:], y_sb[:])

    nc.compile()
    return nc


def _prep_weights(w1, b1, w2, b2):
    import ml_dtypes
    bf = ml_dtypes.bfloat16
    w1f = w1[:, :, 0, 0].astype(np.float32)
    wk, wv = w1f[64:128], w1f[128:192]
    bv = b1[128:192].astype(np.float32)
    w2k = w2[0, 64:128].astype(np.float32)                     # [c,5,5]
    u = np.tensordot(w2k, wk, axes=([0], [0])).reshape(NT, C)  # [25, 64]
    u2 = np.zeros((128, 50), np.float32)
    u2[0:64, 0:25] = u.T
    u2[64:128, 25:50] = u.T
    # s5[(dhp, h_in), (k, h_out)] = 1 iff h_in == h_out + (2k+dhp) - 2
    s5 = np.zeros((2, 64, 3, 64), np.float32)
    for dh in range(5):
        k, dhp = dh // 2, dh % 2
        for ho in range(64):
            hi = ho + dh - 2
            if 0 <= hi < 64:
                s5[dhp, hi, k, ho] = 1.0
    s5 = s5.reshape(128, 3 * 64)
    ident = np.eye(128, dtype=np.float32)
    # wvp[(w'a, c), (w'b, co)] = wv[co, c] * [w'a == w'b]
    wvp = np.zeros((2, 64, 2, 64), np.float32)
    wvp[0, :, 0, :] = wv.T
    wvp[1, :, 1, :] = wv.T
    wvp = wvp.reshape(128, 128)
    bvp = np.concatenate([bv, bv]).reshape(128, 1).astype(np.float32)
    return (u2.astype(bf), s5.astype(bf), ident, wvp.astype(bf), bvp)


def _run(x, w1, b1, w2, b2, trace=False):
    import ml_dtypes
    from concourse.bass_utils import run_bass_kernel_spmd
    bf = ml_dtypes.bfloat16
    if "nc" not in _cache:
        _cache["nc"] = _build_program()
    nc = _cache["nc"]
    u2, s5, ident, wvp, bvp = _prep_weights(w1, b1, w2, b2)
    in_maps = []
    for b in range(B):
        xb = np.ascontiguousarray(
            x[b].reshape(C, HB, HH, W * S).transpose(1, 0, 2, 3)
            .reshape(128, FREE)).astype(bf)
        # x2 [(wb, h), (w', c, t)]
        x2b = np.ascontiguousarray(
            x[b].reshape(C, H, 2, WP, S).transpose(2, 1, 3, 0, 4)
            .reshape(128, FREE)).astype(bf)
        in_maps.append({"x": xb, "x2": x2b, "u2": u2, "s5": s5,
                        "ident": ident, "wvp": wvp, "bvp": bvp})
    res = run_bass_kernel_spmd(nc, in_maps, core_ids=list(range(8)), trace=trace)
    out = np.empty((B, C, H, W, S), np.float32)
    for b in range(B):
        # y [128=(w'a, co), 2048=(jj, wb, h)]
        yb = res.results[b]["y"].reshape(2, C, NTL, 2, H)
        # out[co, h, w = wb*32 + jj*2 + w'a]
        yb = yb.transpose(1, 4, 3, 2, 0).reshape(C, H, W)
        out[b] = yb[..., None]
    return out, res


def kernel(x, w1, b1, w2, b2):
    out, _ = _run(x, w1, b1, w2, b2, trace=False)
    return out

